# revision 1
# baseline (speedup 1.0000x reference)
import sys

sys.path.insert(0, "/opt/trn_rl_repo")

import numpy as np
import concourse.bass as bass  # noqa: F401  (registers types)
from concourse import bacc
import concourse.mybir as mybir
from concourse.tile import TileContext
from concourse.bass_utils import run_bass_kernel_spmd

S = 4096          # sequence length
D = 1024          # model/key/value dim
NCORES = 8
R = S // NCORES   # 512 rows per core
KC = D // 128     # 8 contraction chunks
J = S // 128      # 32 key tiles
VA = D + 2        # V augmented with ones column (denominator) + zero pad (fp32r even-size rule)
CH = [(0, 342), (342, 342), (684, 342)]  # PV output column chunks (<=512 moving, >=256, even)
JQ = 8            # key tiles per PV quarter

F32 = mybir.dt.float32
F32R = mybir.dt.float32r

_cache = {}


def _build_phase1():
    """Per core: q = xs@(Wq/sqrt(D)), k = xs@Wk, v = xs@Wv for its 512-row x slice.

    One weight-load of each x chunk feeds 6 matmuls (3 projections x 2 column
    halves). Biases added on host.
      xsT [128, KC*R]: [p, k*R+i] = x[i, 128k+p]
      wq/wk/wv [128, KC*D]: [p, k*D+d] = W[128k+p, d]
    Outputs: q/k/v [R, D] natural layout.
    """
    nc = bacc.Bacc(None, target_bir_lowering=False)
    xsT = nc.dram_tensor("xsT", [128, KC * R], F32R, kind="ExternalInput")
    wins = [nc.dram_tensor(n, [128, KC * D], F32R, kind="ExternalInput")
            for n in ("wq", "wk", "wv")]
    outs = [nc.dram_tensor(n, [R, D], F32, kind="ExternalOutput") for n in ("q", "k", "v")]
    with TileContext(nc) as tc:
        with tc.tile_pool(name="inp", bufs=1) as inp, \
             tc.tile_pool(name="ob", bufs=6) as ob, \
             tc.tile_pool(name="ps", bufs=6, space="PSUM") as ps:
            xt = inp.tile([128, KC * R], F32R)
            wts = [inp.tile([128, KC * D], F32R, name=f"w{w_i}") for w_i in range(3)]
            # k-interleaved issue order: first matmul needs only the first two DMAs
            for k in range(KC):
                nc.sync.dma_start(xt[:, k * R : (k + 1) * R], xsT[:, k * R : (k + 1) * R])
                nc.sync.dma_start(wts[0][:, k * D : (k + 1) * D], wins[0][:, k * D : (k + 1) * D])
            for w_i in (1, 2):
                for k in range(KC):
                    nc.sync.dma_start(wts[w_i][:, k * D : (k + 1) * D], wins[w_i][:, k * D : (k + 1) * D])
            for w_i in range(3):
                for i in range(R // 128):
                    pz = [ps.tile([128, 512], F32, name=f"p{w_i}_{i}_{n2}", tag="ps")
                          for n2 in range(2)]
                    for k in range(KC):
                        lhsT = xt[:, k * R + i * 128 : k * R + i * 128 + 128]
                        for n2 in range(2):
                            nc.tensor.matmul(
                                pz[n2][:],
                                lhsT,
                                wts[w_i][:, k * D + n2 * 512 : k * D + (n2 + 1) * 512],
                                start=(k == 0), stop=(k == KC - 1),
                            )
                    for n2 in range(2):
                        o = ob.tile([128, 512], F32, name=f"o{w_i}_{i}_{n2}", tag="ob")
                        nc.vector.tensor_copy(o[:], pz[n2][:])
                        nc.sync.dma_start(
                            outs[w_i][i * 128 : (i + 1) * 128, n2 * 512 : (n2 + 1) * 512], o[:]
                        )
    nc.finalize()
    return nc


def _build_phase2():
    """Per core: anti-causal attention for its 512 query rows vs all 4096 keys.

    Scores computed transposed (S^T[j,i], keys on partitions), masked+exp'd via
    an iota<=thr data mask. P^T @ V_aug accumulates over j in PSUM per quarter
    (8 j-tiles), with one P^T weight-load per (i, j) feeding 3 column chunks.
    The ones column of V_aug yields the softmax denominator.
      qt [128, KC*R]: [p, k*R+i] = qT[128k+p, i]   (q pre-scaled by 1/sqrt(D))
      kt [128, J*D]:  [p, j*D + k*128 + c] = kT[128k+p, 128j+c]
      vi [128, J*VA]: [p, j*VA + c] = v_aug[128j+p, c]
      io [128, R]: iota row (0..R-1), th [128, J]: thr[p,j] = 128j+p-512*core
    Output rd [R, D] = normalized attention read.
    """
    nc = bacc.Bacc(None, target_bir_lowering=False)
    qt_in = nc.dram_tensor("qt", [128, KC * R], F32R, kind="ExternalInput")
    kt_in = nc.dram_tensor("kt", [128, J * D], F32R, kind="ExternalInput")
    v_in = nc.dram_tensor("vi", [128, J * VA], F32R, kind="ExternalInput")
    iota = nc.dram_tensor("io", [128, R], F32, kind="ExternalInput")
    thr = nc.dram_tensor("th", [128, J], F32, kind="ExternalInput")
    rdT = nc.dram_tensor("rdT", [D + 128, R], F32, kind="ExternalOutput")
    NN = D // 128  # 8 output feature chunks
    NQ = J // JQ   # 4 quarters
    with TileContext(nc) as tc:
        with tc.tile_pool(name="cst", bufs=1) as cst, \
             tc.tile_pool(name="kp", bufs=3) as kp, \
             tc.tile_pool(name="sp", bufs=2, space="PSUM") as sp, \
             tc.tile_pool(name="ep", bufs=3) as ep, \
             tc.tile_pool(name="pp", bufs=2 * JQ) as ppool, \
             tc.tile_pool(name="vp", bufs=2 * JQ) as vp, \
             tc.tile_pool(name="p2", bufs=6, space="PSUM") as p2, \
             tc.tile_pool(name="ac", bufs=NN + 1) as ac, \
             tc.tile_pool(name="no", bufs=5) as no:
            qt = cst.tile([128, KC * R], F32R)
            nc.sync.dma_start(qt[:, 0:R], qt_in[:, 0:R])
            io = cst.tile([128, R], F32)
            nc.sync.dma_start(io[:], iota[:])
            th = cst.tile([128, J], F32)
            nc.sync.dma_start(th[:], thr[:])
            # ---- per quarter: scores+exp+mask for 8 j-tiles, then PV ----
            pts = {}
            accs = {}
            for q in range(NQ):
                for jj in range(JQ):
                    j = q * JQ + jj
                    kt = kp.tile([128, D], F32R, name=f"kt{j}", tag="kt")
                    nc.sync.dma_start(kt[:], kt_in[:, j * D : (j + 1) * D])
                    if j == 0:
                        # remaining q chunks ride behind the first key tile so the
                        # first matmul only waits for ~1MB, not the whole q load
                        for k in range(1, KC):
                            nc.sync.dma_start(
                                qt[:, k * R : (k + 1) * R], qt_in[:, k * R : (k + 1) * R]
                            )
                    ps_ = sp.tile([128, R], F32, name=f"s{j}", tag="s")
                    for k in range(KC):
                        nc.tensor.matmul(
                            ps_[:],
                            kt[:, k * 128 : (k + 1) * 128],
                            qt[:, k * R : (k + 1) * R],
                            start=(k == 0), stop=(k == KC - 1),
                        )
                    ex = ep.tile([128, R], F32, name=f"e{j}", tag="e")
                    nc.scalar.activation(ex[:], ps_[:], mybir.ActivationFunctionType.Exp)
                    pt = ppool.tile([128, R], F32R, name=f"pt{j}", tag="pt")
                    nc.vector.scalar_tensor_tensor(
                        pt[:], io[:], th[:, j : j + 1], ex[:],
                        op0=mybir.AluOpType.is_le, op1=mybir.AluOpType.mult,
                    )
                    pts[j] = pt
                vts = []
                for jj in range(JQ):
                    j = q * JQ + jj
                    vt = vp.tile([128, VA], F32R, name=f"vt{j}", tag="vt")
                    nc.sync.dma_start(vt[:], v_in[:, j * VA : (j + 1) * VA])
                    vts.append(vt)
                for n in range(NN + 1):  # 8 feature chunks + (ones, pad) chunk
                    c0, w = (n * 128, 128) if n < NN else (D, 2)
                    pz = p2.tile([128, R], F32, name=f"pv{q}_{n}", tag="pv")
                    for jj in range(JQ):
                        j = q * JQ + jj
                        nc.tensor.matmul(
                            pz[:w, :],
                            vts[jj][:, c0 : c0 + w],
                            pts[j][:],
                            start=(jj == 0), stop=(jj == JQ - 1),
                        )
                    if q == 0:
                        a_ = ac.tile([128, R], F32, name=f"acc{n}", tag="ac")
                        accs[n] = a_
                        nc.vector.tensor_copy(a_[:w, :], pz[:w, :])
                    else:
                        a_ = accs[n]
                        nc.vector.tensor_add(a_[:w, :], a_[:w, :], pz[:w, :])
            # ---- ship unnormalized read^T + denominator row; host divides ----
            for n in range(NN):
                nc.sync.dma_start(rdT[n * 128 : (n + 1) * 128, :], accs[n][:])
            nc.sync.dma_start(rdT[D : D + 2, :], accs[NN][:2, :])
    nc.finalize()
    return nc


def _chunk_rows(a, nchunks):
    # [nchunks*128, C] -> [128, nchunks*C] with [p, k*C+c] = a[128k+p, c]
    n, c = a.shape
    assert n == nchunks * 128
    return np.ascontiguousarray(
        a.reshape(nchunks, 128, c).transpose(1, 0, 2).reshape(128, nchunks * c)
    )


def kernel(x, Wk, bk, Wq, bq, Wv, bv):
    x = np.asarray(x, dtype=np.float32)
    Wk = np.asarray(Wk, dtype=np.float32)
    Wq = np.asarray(Wq, dtype=np.float32)
    Wv = np.asarray(Wv, dtype=np.float32)
    bk = np.asarray(bk, dtype=np.float32)
    bq = np.asarray(bq, dtype=np.float32)
    bv = np.asarray(bv, dtype=np.float32)

    sc = np.float32(1.0 / np.sqrt(D))
    if "p1" not in _cache:
        _cache["p1"] = _build_phase1()
    if "p2" not in _cache:
        _cache["p2"] = _build_phase2()

    wq_in = _chunk_rows(Wq * sc, KC)
    wk_in = _chunk_rows(Wk, KC)
    wv_in = _chunk_rows(Wv, KC)
    in_maps1 = []
    for c in range(NCORES):
        xs = x[c * R : (c + 1) * R]
        xsT_in = _chunk_rows(np.ascontiguousarray(xs.T), KC)
        in_maps1.append({"xsT": xsT_in, "wq": wq_in, "wk": wk_in, "wv": wv_in})
    res1 = run_bass_kernel_spmd(_cache["p1"], in_maps1, list(range(NCORES))).results

    bq_s = (bq * sc)[None, :]
    qs = [res1[c]["q"] + bq_s for c in range(NCORES)]
    k_g = np.concatenate([res1[c]["k"] for c in range(NCORES)], axis=0) + bk[None, :]
    v_g = np.concatenate([res1[c]["v"] for c in range(NCORES)], axis=0) + bv[None, :]
    kT_g = np.ascontiguousarray(k_g.T)  # [D, S]
    v_aug = np.concatenate(
        [v_g, np.ones((S, 1), np.float32), np.zeros((S, 1), np.float32)], axis=1
    )

    # kt layout: [p, j, k, c] = kT_g[128k+p, 128j+c]
    kt_in = np.ascontiguousarray(
        kT_g.reshape(KC, 128, J, 128).transpose(1, 2, 0, 3).reshape(128, J * D)
    )
    v_in = _chunk_rows(v_aug, J)
    io_in = np.ascontiguousarray(
        np.broadcast_to(np.arange(R, dtype=np.float32), (128, R))
    )
    p_idx = np.arange(128, dtype=np.float32)[:, None]
    j_idx = np.arange(J, dtype=np.float32)[None, :]
    in_maps2 = []
    for c in range(NCORES):
        thr_c = np.ascontiguousarray(128.0 * j_idx + p_idx - 512.0 * c).astype(np.float32)
        in_maps2.append({
            "qt": _chunk_rows(np.ascontiguousarray(qs[c].T), KC),
            "kt": kt_in,
            "vi": v_in,
            "io": io_in,
            "th": thr_c,
        })
    res2 = run_bass_kernel_spmd(_cache["p2"], in_maps2, list(range(NCORES))).results

    read = np.concatenate(
        [(res2[c]["rdT"][:D] / res2[c]["rdT"][D : D + 1]).T for c in range(NCORES)], axis=0
    )
    return np.concatenate([x, read], axis=1)



# revision 7
# speedup vs baseline: 1.6516x; 1.6516x over previous
import sys

sys.path.insert(0, "/opt/trn_rl_repo")

import math

import ml_dtypes
import numpy as np
import concourse.bass as bass  # noqa: F401  (registers types)
from concourse import bacc
import concourse.mybir as mybir
from concourse.tile import TileContext
from concourse.bass_utils import run_bass_kernel_spmd

S = 4096          # sequence length
D = 1024          # model/key/value dim
NCORES = 8
R = S // NCORES   # 512 query rows per core
KC = D // 128     # 8 contraction chunks
NF = D // 128     # 8 feature chunks
J = S // 128      # 32 key tiles
VA = D + 16       # V augmented with ones column (denominator) + zero pad
                  # (pad to %16==0: DoubleRow weight APs need dim1 step%16==0)
SC = 1.0 / math.sqrt(D)

F32 = mybir.dt.float32
BF16 = mybir.dt.bfloat16
F8 = mybir.dt.float8e4
DR = mybir.MatmulPerfMode.DoubleRow
NP_F8 = ml_dtypes.float8_e4m3
NP_BF16 = ml_dtypes.bfloat16

_cache = {}
# test.py can flip TRACE to get exec_time_ns of the two launches in LAST_NS
TRACE = False
LAST_NS = None


def _build_phase1():
    """Per core: q^T, k^T (transposed, contraction-chunked) and v (natural) for
    its 512-row x slice, all fp8 DoubleRow matmuls (2 contraction chunks of 128
    per instruction).

      xsT [128, KC, R]: [p, k, i] = x[rows[i], 128k+p]
      wq/wk [128, NF*KC, 128]: [p, f*KC+k, c] = W[128k+p, 128f+c]  (feature-major)
      wv [128, KC, D]: [p, k, d] = W[128k+p, d]
    Outputs (bias added on host):
      qT/kT [128, KC, R] bf16: [p, k, i] = (x@W)^T[128k+p, i]
      vO [R, D] bf16 natural.
    """
    nc = bacc.Bacc(None, target_bir_lowering=False)
    xsT = nc.dram_tensor("xsT", [128, KC, R], F8, kind="ExternalInput")
    wq = nc.dram_tensor("wq", [128, NF * KC, 128], F8, kind="ExternalInput")
    wk = nc.dram_tensor("wk", [128, NF * KC, 128], F8, kind="ExternalInput")
    wv = nc.dram_tensor("wv", [128, KC, D], F8, kind="ExternalInput")
    qT = nc.dram_tensor("qT", [128, KC, R], BF16, kind="ExternalOutput")
    kT = nc.dram_tensor("kT", [128, KC, R], BF16, kind="ExternalOutput")
    vO = nc.dram_tensor("vO", [R, D], BF16, kind="ExternalOutput")
    with TileContext(nc) as tc:
        with tc.tile_pool(name="inp", bufs=1) as inp, \
             tc.tile_pool(name="ob", bufs=6) as ob, \
             tc.tile_pool(name="ps", bufs=4, space="PSUM") as ps:
            xt = inp.tile([128, KC, R], F8)
            wqt = inp.tile([128, NF * KC, 128], F8)
            wkt = inp.tile([128, NF * KC, 128], F8)
            wvt = inp.tile([128, KC, D], F8)
            nc.sync.dma_start(xt[:], xsT[:])
            for f in range(NF):
                nc.sync.dma_start(wqt[:, f * KC:(f + 1) * KC, :], wq[:, f * KC:(f + 1) * KC, :])
            for f in range(NF):
                nc.sync.dma_start(wkt[:, f * KC:(f + 1) * KC, :], wk[:, f * KC:(f + 1) * KC, :])
            for k in range(KC):
                nc.sync.dma_start(wvt[:, k, :], wv[:, k, :])
            for w_i, (wt, outT) in enumerate(((wqt, qT), (wkt, kT))):
                for f in range(NF):
                    pz = ps.tile([128, R], F32, name=f"pz{w_i}_{f}", tag="ps")
                    for h in range(2):
                        for kk in range(KC // 2):
                            nc.tensor.matmul(
                                pz[:, h * 256:(h + 1) * 256],
                                wt[:, f * KC + 2 * kk: f * KC + 2 * kk + 2, :],
                                xt[:, 2 * kk:2 * kk + 2, h * 256:h * 256 + 256],
                                start=(kk == 0), stop=(kk == KC // 2 - 1),
                                perf_mode=DR,
                            )
                    o = ob.tile([128, R], BF16, name=f"o{w_i}_{f}", tag="ob")
                    if f % 2 == 0:
                        nc.vector.tensor_copy(o[:], pz[:])
                    else:
                        nc.scalar.copy(o[:], pz[:])
                    nc.sync.dma_start(outT[:, f, :], o[:])
            for i in range(R // 128):
                for fh in range(2):
                    pz = ps.tile([128, R], F32, name=f"pv{i}_{fh}", tag="ps")
                    for q2 in range(2):
                        for kk in range(KC // 2):
                            nc.tensor.matmul(
                                pz[:, q2 * 256:(q2 + 1) * 256],
                                xt[:, 2 * kk:2 * kk + 2, i * 128:(i + 1) * 128],
                                wvt[:, 2 * kk:2 * kk + 2,
                                    fh * 512 + q2 * 256: fh * 512 + q2 * 256 + 256],
                                start=(kk == 0), stop=(kk == KC // 2 - 1),
                                perf_mode=DR,
                            )
                    o = ob.tile([128, R], BF16, name=f"ov{i}_{fh}", tag="ob")
                    if fh == 0:
                        nc.vector.tensor_copy(o[:], pz[:])
                    else:
                        nc.scalar.copy(o[:], pz[:])
                    nc.sync.dma_start(vO[i * 128:(i + 1) * 128, fh * 512:(fh + 1) * 512], o[:])
    nc.finalize()
    return nc


def _build_phase2():
    """Per core: anti-causal attention for its 512 query rows vs all 4096 keys,
    fp8 DoubleRow throughout.

    Stage 1 per key tile j: scores^T [128 keys, 512 q] accumulated over 4
    contraction pairs, exp(scale*s) on the activation engine, then masked into
    an fp8 P^T pair tile (j-pairs stacked on dim1 for DoubleRow PV).
    Stage 2 per feature chunk: PV accumulates all 16 j-pairs in PSUM; the ones
    column of V_aug yields the softmax denominator.

      qt [128, KC, R] fp8: [p, k, i] = (q+bq)^T[128k+p, i]   (unscaled)
      kt [128, J*KC, 128] fp8: [p, j*KC+k, c] = (k+bk)^T[128k+p, 128j+c]
      vi [128, J, VA] fp8: [p, j, c] = v_aug[128j+p, c]
      io [128, R] f32: [p, i] = i;  th [128, J] f32: [p, j] = 128j+p-512*core
    Outputs: rdT [NF, 128, R] bf16 numerators, dn [2, R] bf16 (row 0 = denom).
    """
    nc = bacc.Bacc(None, target_bir_lowering=False)
    qt = nc.dram_tensor("qt", [128, KC, R], F8, kind="ExternalInput")
    kt = nc.dram_tensor("kt", [128, J * KC, 128], F8, kind="ExternalInput")
    vi = nc.dram_tensor("vi", [128, J, VA], F8, kind="ExternalInput")
    io = nc.dram_tensor("io", [128, R], F32, kind="ExternalInput")
    th = nc.dram_tensor("th", [128, J], F32, kind="ExternalInput")
    rdT = nc.dram_tensor("rdT", [NF, 128, R], BF16, kind="ExternalOutput")
    dn = nc.dram_tensor("dn", [2, R], BF16, kind="ExternalOutput")
    with TileContext(nc) as tc:
        with tc.tile_pool(name="cst", bufs=1) as cst, \
             tc.tile_pool(name="kp", bufs=4) as kp, \
             tc.tile_pool(name="vp", bufs=J // 2) as vp, \
             tc.tile_pool(name="sp", bufs=2, space="PSUM") as sp, \
             tc.tile_pool(name="ep", bufs=3) as ep, \
             tc.tile_pool(name="pp", bufs=J // 2) as pp, \
             tc.tile_pool(name="p2", bufs=3, space="PSUM") as p2, \
             tc.tile_pool(name="no", bufs=4) as no:
            iot = cst.tile([128, R], F32)
            tht = cst.tile([128, J], F32)
            qtt = cst.tile([128, KC, R], F8)
            nc.sync.dma_start(iot[:], io[:])
            nc.sync.dma_start(tht[:], th[:])
            nc.sync.dma_start(qtt[:], qt[:])
            pts = [pp.tile([128, 2, R], F8, name=f"pt{t}", tag="pt")
                   for t in range(J // 2)]
            for j in range(J):
                ktj = kp.tile([128, KC, 128], F8, name=f"kt{j}", tag="kt")
                nc.sync.dma_start(ktj[:], kt[:, j * KC:(j + 1) * KC, :])
                s = sp.tile([128, R], F32, name=f"s{j}", tag="s")
                for h in range(2):
                    for kk in range(KC // 2):
                        nc.tensor.matmul(
                            s[:, h * 256:(h + 1) * 256],
                            ktj[:, 2 * kk:2 * kk + 2, :],
                            qtt[:, 2 * kk:2 * kk + 2, h * 256:h * 256 + 256],
                            start=(kk == 0), stop=(kk == KC // 2 - 1),
                            perf_mode=DR,
                        )
                ex = ep.tile([128, R], F32, name=f"e{j}", tag="e")
                nc.scalar.activation(ex[:], s[:], mybir.ActivationFunctionType.Exp,
                                     scale=SC)
                nc.vector.scalar_tensor_tensor(
                    pts[j // 2][:, j % 2, :], iot[:], tht[:, j:j + 1], ex[:],
                    op0=mybir.AluOpType.is_le, op1=mybir.AluOpType.mult,
                )
            vtp = [vp.tile([128, 2, VA], F8, name=f"vt{t}", tag="vt")
                   for t in range(J // 2)]
            for t in range(J // 2):
                nc.sync.dma_start(vtp[t][:], vi[:, 2 * t:2 * t + 2, :])
            for n in range(NF + 1):
                c0, w = (n * 128, 128) if n < NF else (D, 2)
                pz = p2.tile([128, R], F32, name=f"pv{n}", tag="pv")
                for h in range(2):
                    for t in range(J // 2):
                        nc.tensor.matmul(
                            pz[:w, h * 256:(h + 1) * 256],
                            vtp[t][:, :, c0:c0 + w],
                            pts[t][:, :, h * 256:h * 256 + 256],
                            start=(t == 0), stop=(t == J // 2 - 1),
                            perf_mode=DR,
                        )
                o = no.tile([128, R], BF16, name=f"no{n}", tag="no")
                if n % 2 == 0:
                    nc.scalar.copy(o[:w, :], pz[:w, :])
                else:
                    nc.vector.tensor_copy(o[:w, :], pz[:w, :])
                if n < NF:
                    nc.sync.dma_start(rdT[n, :, :], o[:])
                else:
                    nc.sync.dma_start(dn[:, :], o[:2, :])
    nc.finalize()
    return nc


def _f8(a):
    return np.asarray(a, dtype=NP_F8)


def prep_phase1_inputs(x, Wq, Wk, Wv):
    """Build per-core phase-1 input maps (all fp8)."""
    wq_in = _f8(Wq.reshape(KC, 128, NF, 128).transpose(1, 2, 0, 3)
                .reshape(128, NF * KC, 128))
    wk_in = _f8(Wk.reshape(KC, 128, NF, 128).transpose(1, 2, 0, 3)
                .reshape(128, NF * KC, 128))
    wv_in = _f8(Wv.reshape(KC, 128, D).transpose(1, 0, 2))
    in_maps = []
    for c in range(NCORES):
        xs = x[c * R:(c + 1) * R]                      # [R, D]
        xsT = _f8(xs.T.reshape(KC, 128, R).transpose(1, 0, 2))  # [p, k, i]
        in_maps.append({"xsT": xsT, "wq": wq_in, "wk": wk_in, "wv": wv_in})
    return in_maps


def prep_phase2_inputs(res1, bq, bk, bv):
    """Combine phase-1 outputs + biases into per-core phase-2 input maps."""
    bias_pk_q = bq.reshape(KC, 128).T                  # [p, k]
    bias_pk_k = bk.reshape(KC, 128).T
    kT_full = np.concatenate(
        [np.asarray(res1[c]["kT"], dtype=np.float32) for c in range(NCORES)], axis=2
    ) + bias_pk_k[:, :, None]                          # [128, KC, S]
    kt_in = _f8(kT_full.reshape(128, KC, J, 128).transpose(0, 2, 1, 3)
                .reshape(128, J * KC, 128))
    v_full = np.concatenate(
        [np.asarray(res1[c]["vO"], dtype=np.float32) for c in range(NCORES)], axis=0
    ) + bv[None, :]                                    # [S, D]
    v_aug = np.concatenate(
        [v_full, np.ones((S, 1), np.float32), np.zeros((S, VA - D - 1), np.float32)],
        axis=1,
    )
    vi_in = _f8(v_aug.reshape(J, 128, VA).transpose(1, 0, 2))
    io_in = np.ascontiguousarray(
        np.broadcast_to(np.arange(R, dtype=np.float32), (128, R))
    )
    p_idx = np.arange(128, dtype=np.float32)[:, None]
    j_idx = np.arange(J, dtype=np.float32)[None, :]
    in_maps = []
    for c in range(NCORES):
        qt_c = np.asarray(res1[c]["qT"], dtype=np.float32) + bias_pk_q[:, :, None]
        thr_c = np.ascontiguousarray(128.0 * j_idx + p_idx - 512.0 * c).astype(np.float32)
        in_maps.append({
            "qt": _f8(qt_c), "kt": kt_in, "vi": vi_in, "io": io_in, "th": thr_c,
        })
    return in_maps


def finish_output(x, res2):
    read = np.concatenate(
        [
            (np.asarray(res2[c]["rdT"], dtype=np.float32).reshape(D, R)
             / np.asarray(res2[c]["dn"], dtype=np.float32)[0:1, :]).T
            for c in range(NCORES)
        ],
        axis=0,
    )
    return np.concatenate([x, read], axis=1).astype(np.float32)


def kernel(x, Wk, bk, Wq, bq, Wv, bv):
    global LAST_NS
    x = np.asarray(x, dtype=np.float32)
    Wk = np.asarray(Wk, dtype=np.float32)
    Wq = np.asarray(Wq, dtype=np.float32)
    Wv = np.asarray(Wv, dtype=np.float32)
    bk = np.asarray(bk, dtype=np.float32)
    bq = np.asarray(bq, dtype=np.float32)
    bv = np.asarray(bv, dtype=np.float32)

    if "p1" not in _cache:
        _cache["p1"] = _build_phase1()
    if "p2" not in _cache:
        _cache["p2"] = _build_phase2()

    in_maps1 = prep_phase1_inputs(x, Wq, Wk, Wv)
    r1 = run_bass_kernel_spmd(_cache["p1"], in_maps1, list(range(NCORES)), trace=TRACE)
    in_maps2 = prep_phase2_inputs(r1.results, bq, bk, bv)
    r2 = run_bass_kernel_spmd(_cache["p2"], in_maps2, list(range(NCORES)), trace=TRACE)
    if TRACE and r1.exec_time_ns and r2.exec_time_ns:
        LAST_NS = int(r1.exec_time_ns + r2.exec_time_ns)
    return finish_output(x, r2.results)
